# revision 28
# baseline (speedup 1.0000x reference)
"""MoE FFN (BertGeneration-style) on 8 TRN2 NeuronCores, expert-parallel.

Problem: 8192 tokens, expert = task_id % 8, per-expert FFN
(768 -> 3072 gelu -> 768) + residual + per-expert LayerNorm.

Strategy: routing (dispatch/combine) is a host-side permutation; each of the
8 cores runs one expert's FFN over its 1024-token block.  Both GEMMs run in
fp8 e4m3 with the PE's DoubleRow perf mode (two contraction rows packed per
partition -> K=256 per matmul, 2x fp32r throughput).  Power-of-2 scales keep
the fp8 payloads in e4m3's normal range:
  x*32, W1*4096  -> gelu dequants via activation scale 2^-17 (exact)
  W2*8192        -> cancels in LayerNorm (scale-invariant) by pre-scaling
                    the residual (x+b2)*8192 on host; no extra on-chip ops.
On-chip:
  phase 1:  hT[i, m] = gelu(psum * 2^-17 + b1) as fp8   (h transposed)
  phase 2:  y[m, h]  = sum_i hT[i, m] * W2[i, h];  z = y + 8192*(x + b2);
            LayerNorm(z) along h.
"""

import sys

if "/opt/trn_rl_repo" not in sys.path:
    sys.path.insert(0, "/opt/trn_rl_repo")

import numpy as np


def _install_axon_hooks_shim():
    """Provide antenv.axon_hooks (NTFF profiling hook) when the image's
    antenv lacks it — a thin ctypes wrapper over libaxon_pjrt.so, matching
    trn_agent_boot.trn_boot._ntff_profile_via_ctypes.  Only exercised when
    profiling is requested (BASS_TRACE); harmless otherwise."""
    import contextlib
    import ctypes
    import types

    try:
        import antenv.axon_hooks  # noqa: F401
        return
    except ImportError:
        pass
    try:
        import antenv
    except ImportError:
        return

    mod = types.ModuleType("antenv.axon_hooks")
    _state = {"hook": None, "init": False}

    def set_axon_ntff_profile_hook(h):
        _state["hook"] = h
        _state["init"] = True

    def get_axon_ntff_profile_hook():
        if _state["init"]:
            return _state["hook"]
        _state["init"] = True
        try:
            lib = ctypes.CDLL("/opt/axon/libaxon_pjrt.so")
        except OSError:
            return None
        if not hasattr(lib, "axon_start_nrt_profile"):
            return None
        lib.axon_start_nrt_profile.argtypes = [
            ctypes.POINTER(ctypes.c_int64), ctypes.c_size_t]
        lib.axon_start_nrt_profile.restype = ctypes.c_int64
        lib.axon_stop_nrt_profile.argtypes = [ctypes.c_char_p]
        lib.axon_stop_nrt_profile.restype = ctypes.c_int64

        @contextlib.contextmanager
        def _hook(output_dir, device_ids):
            import jax
            jax.devices()
            if device_ids:
                ids = (ctypes.c_int64 * len(device_ids))(*device_ids)
                rc = lib.axon_start_nrt_profile(ids, len(device_ids))
            else:
                rc = lib.axon_start_nrt_profile(None, 0)
            if rc != 0:
                raise RuntimeError(f"axon_start_nrt_profile rc={rc}")
            try:
                yield
            finally:
                n = lib.axon_stop_nrt_profile(str(output_dir).encode())
                print(f"profile: {n} file(s) written to {output_dir}")

        _state["hook"] = _hook
        return _hook

    mod.set_axon_ntff_profile_hook = set_axon_ntff_profile_hook
    mod.get_axon_ntff_profile_hook = get_axon_ntff_profile_hook
    sys.modules["antenv.axon_hooks"] = mod
    antenv.axon_hooks = mod


_install_axon_hooks_shim()

E = 8
N = 8192
H = 768
I = 3072
C = N // E        # 1024 tokens per expert/core
KT8 = H // 256    # 3   k-tiles (hidden dim, DoubleRow: 256 per matmul)
IT = I // 128     # 24  i-tiles (intermediate dim)
IT8 = I // 256    # 12  GEMM2 k-tiles (DoubleRow)
MT = C // 128     # 8   m-tiles (token dim per core)
WC = 12           # w1 DMA chunks (2 i-tiles each)

# power-of-2 fp8 scales (e4m3 max 240; values land in the normal range)
SX = 32.0
SW1 = 4096.0
SW2 = 8192.0
INV1 = 1.0 / (SX * SW1)   # dequant folded into the gelu activation

_CACHE = {}


def _build_nc(act_name="Gelu"):
    from contextlib import ExitStack

    import concourse.tile as tile
    from concourse import bacc, mybir

    f32 = mybir.dt.float32
    f8 = mybir.dt.float8e4
    AF = mybir.ActivationFunctionType
    act_fn = getattr(AF, act_name)
    ALU = mybir.AluOpType
    DR = mybir.MatmulPerfMode.DoubleRow

    nc = bacc.Bacc("TRN2", target_bir_lowering=False, debug=False, num_devices=8)

    # fp8 operands: [partition, ...] with DoubleRow pairs as an explicit dim
    xT8 = nc.dram_tensor("xT8", [128, KT8, 2, C], f8, kind="ExternalInput").ap()
    w18 = nc.dram_tensor("w18", [128, WC, 2, KT8, 2, 128], f8,
                         kind="ExternalInput").ap()
    w28 = nc.dram_tensor("w28", [128, IT8, 2, H], f8, kind="ExternalInput").ap()
    b1t = nc.dram_tensor("b1t", [128, IT], f32, kind="ExternalInput").ap()
    xn = nc.dram_tensor("xn", [128, MT, H], f32, kind="ExternalInput").ap()
    out = nc.dram_tensor("out", [128, MT, H], f32, kind="ExternalOutput").ap()

    with ExitStack() as ctx:
        tc = ctx.enter_context(tile.TileContext(nc))
        persist = ctx.enter_context(tc.tile_pool(name="persist", bufs=1))
        psum = ctx.enter_context(tc.tile_pool(name="psum", bufs=4, space="PSUM"))
        w1pool = ctx.enter_context(tc.tile_pool(name="w1s", bufs=6))
        spool = ctx.enter_context(tc.tile_pool(name="small", bufs=4))
        zpool = ctx.enter_context(tc.tile_pool(name="zs", bufs=3))

        hT = persist.tile([128, IT, C], f8, name="hT")
        w2f = [persist.tile([128, 4, 2, H], f8, name=f"w2c{j}", tag=f"w2c{j}")
               for j in range(IT8 // 4)]
        xk0a = persist.tile([128, 2, 512], f8, name="xk0a")
        xk0b = persist.tile([128, 2, 512], f8, name="xk0b")
        xk12 = persist.tile([128, 2, 2, C], f8, name="xk12")
        xnh = [persist.tile([128, 4, H], f32, name=f"xnh{j}", tag=f"xnh{j}")
               for j in range(2)]
        b1s = persist.tile([128, IT], f32, name="b1s")
        epsT = persist.tile([128, 1], f32, name="epsT")
        # PE/act warm-up scratch (zeros; results never consumed)
        wuL = persist.tile([128, 2, 128], f8, name="wuL")
        wuR = persist.tile([128, 2, 512], f8, name="wuR")
        wuO = persist.tile([128, 1], f32, name="wuO")

        nc.gpsimd.memset(wuL, 0.0)
        nc.gpsimd.memset(wuR, 0.0)
        nc.vector.memset(epsT, 1e-12)
        # load the Gelu act table during the startup-DMA window instead of
        # lazily at the first real activation
        nc.scalar.activation(wuO, epsT, act_fn, bias=epsT)

        def rhs1(t, half):
            if t == 0:
                return xk0a if half == 0 else xk0b
            return xk12[:, t - 1][:, :, half * 512:(half + 1) * 512]

        # ---- startup DMAs: both queues stream the PE-critical chunks
        # first (w1 even chunks on sync, odd on gpsimd); w2/xn queue behind
        # them so ring contention can't starve the phase-1 weight stream
        w1ts = {}

        # The w1 chunk stream — PE-critical — rides the sync hardware DMA
        # ring alone (issue rate 0.65us/chunk vs 2.6us/chunk consumption).
        # The gpsimd ring is Q7-software-driven (~2-3us per descriptor) and
        # gets the early-small (xk, b1) and high-slack (w2) transfers.
        def load_w1(c):
            w1t = w1pool.tile([128, 2, KT8, 2, 128], f8, name="w1t", tag="w1t")
            nc.sync.dma_start(out=w1t, in_=w18[:, c])
            w1ts[c] = w1t

        nc.gpsimd.dma_start(out=xk0a, in_=xT8[:, 0, :, 0:512])
        load_w1(0)
        nc.sync.dma_start(out=xk12, in_=xT8[:, 1:3])
        nc.gpsimd.dma_start(out=xk0b, in_=xT8[:, 0, :, 512:1024])
        nc.gpsimd.dma_start(out=b1s, in_=b1t)
        load_w1(1)
        load_w1(2)
        load_w1(3)

        # ramp the PE clock out of its low pstate while the startup DMAs
        # are in flight (zero-input matmuls; psum result unread)
        wuP = psum.tile([128, C], f32, name="ph", tag="pt")
        for _ in range(4):
            nc.tensor.matmul(wuP[:, 0:512], lhsT=wuL, rhs=wuR, start=True,
                             stop=True, perf_mode=DR)

        # ---- phase 1: hT = gelu((W1.T @ xT) * 2^-17 + b1) as fp8 ----
        # warm-up block: first NW i-tiles processed kt-major so the PE can
        # start on xk chunk 0 while chunks 1-2 are still in flight
        NW = 3
        load_w1(4)
        load_w1(5)
        phs = [psum.tile([128, C], f32, name="ph", tag="pt") for _ in range(NW)]
        for t in range(KT8):
            for it in range(NW):
                lhsT = w1ts[it // 2][:, it % 2, t]
                for half in range(2):
                    nc.tensor.matmul(
                        phs[it][:, half * 512:(half + 1) * 512],
                        lhsT=lhsT,
                        rhs=rhs1(t, half),
                        start=(t == 0),
                        stop=(t == KT8 - 1),
                        perf_mode=DR,
                    )
        for it in range(NW):
            nc.scalar.activation(hT[:, it, :], phs[it], act_fn,
                                 bias=b1s[:, it:it + 1], scale=INV1)

        for it in range(NW, IT):
            if it % 2 == 0 and it // 2 + 4 < WC:
                load_w1(it // 2 + 4)
            ph = psum.tile([128, C], f32, name="ph", tag="pt")
            lhs_c = w1ts[it // 2]
            for t in range(KT8):
                lhsT = lhs_c[:, it % 2, t]
                for half in range(2):
                    nc.tensor.matmul(
                        ph[:, half * 512:(half + 1) * 512],
                        lhsT=lhsT,
                        rhs=rhs1(t, half),
                        start=(t == 0),
                        stop=(t == KT8 - 1),
                        perf_mode=DR,
                    )
            nc.scalar.activation(hT[:, it, :], ph, act_fn,
                                 bias=b1s[:, it:it + 1], scale=INV1)

        # w2/xn have tens of us of slack before phase 2: w2 behind the xk
        # stream on the slow gpsimd ring, xn behind the w1 chunks on sync
        for j in range(IT8 // 4):
            nc.gpsimd.dma_start(out=w2f[j], in_=w28[:, 4 * j:4 * j + 4])
        for j in range(2):
            nc.sync.dma_start(out=xnh[j], in_=xn[:, 4 * j:4 * j + 4])

        # ---- phase 2: y = hT.T @ W2; z = y + 8192*(x+b2); LayerNorm ----
        for mt in range(MT):
            py = psum.tile([128, C], f32, name="py", tag="pt")
            for t in range(IT8):
                lhsT = hT[:, 2 * t:2 * t + 2, mt * 128:(mt + 1) * 128]
                rhsc = w2f[t // 4][:, t % 4]
                nc.tensor.matmul(
                    py[:, 0:512], lhsT=lhsT, rhs=rhsc[:, :, 0:512],
                    start=(t == 0), stop=(t == IT8 - 1), perf_mode=DR)
                nc.tensor.matmul(
                    py[:, 512:768], lhsT=lhsT, rhs=rhsc[:, :, 512:768],
                    start=(t == 0), stop=(t == IT8 - 1), perf_mode=DR)
            # residual + LayerNorm (gpsimd cannot read PSUM, so the add
            # stays on vector; the normalize halves split across engines)
            z = zpool.tile([128, H], f32, name="z", tag="z")
            xr = xnh[mt // 4][:, mt % 4]
            HH = H // 2
            nc.vector.tensor_add(z, py[:, 0:H], xr)
            stats = spool.tile([128, 2, 6], f32, name="stats", tag="stats")
            nc.vector.bn_stats(stats[:, 0], z[:, 0:HH])
            nc.vector.bn_stats(stats[:, 1], z[:, HH:H])
            mv = spool.tile([128, 2], f32, name="mv", tag="mv")
            nc.vector.bn_aggr(mv, stats)
            rstd = spool.tile([128, 1], f32, name="rstd", tag="rstd")
            nc.scalar.activation(rstd, mv[:, 1:2], AF.Sqrt, bias=epsT)
            nc.vector.reciprocal(out=rstd, in_=rstd)
            for h0 in (0, HH):
                sl = slice(h0, h0 + HH)
                nc.vector.tensor_scalar(
                    out=z[:, sl], in0=z[:, sl], scalar1=mv[:, 0:1],
                    scalar2=rstd, op0=ALU.subtract, op1=ALU.mult)
                nc.scalar.dma_start(out=out[:, mt, sl], in_=z[:, sl])

    nc.compile()
    return nc


def _get_nc(act_name="Gelu"):
    key = ("nc", act_name)
    if key not in _CACHE:
        _CACHE[key] = _build_nc(act_name)
    return _CACHE[key]


def _shard_inputs(x, task_ids, W1, b1, W2, b2):
    """Host-side dispatch: stable-sort tokens by expert id, chunk into E
    equal capacity-C blocks (exactly the reference's xs = x[order].reshape),
    then pack/quantize the fp8 operand layouts."""
    import ml_dtypes

    f8 = ml_dtypes.float8_e4m3
    expert = (task_ids.astype(np.int64) % E).astype(np.int32)
    order = np.argsort(expert, kind="stable")
    xs = x[order]
    in_maps = []
    for e in range(E):
        xe = xs[e * C:(e + 1) * C]                       # [C, H]
        # xT8[p, t, j, c] = xe[c, t*256 + j*128 + p] * SX
        xT8 = (xe.T * SX).reshape(KT8, 2, 128, C).transpose(2, 0, 1, 3)
        # w18[p, chunk, itc, t, j, m] = W1[t*256+j*128+p, (chunk*2+itc)*128+m]
        w18 = (W1[e] * SW1).reshape(KT8, 2, 128, WC, 2, 128).transpose(
            2, 3, 4, 0, 1, 5)
        # w28[p, t, j, h] = W2[t*256+j*128+p, h] * SW2
        w28 = (W2[e] * SW2).reshape(IT8, 2, 128, H).transpose(2, 0, 1, 3)
        b1t = b1[e].reshape(IT, 128).T
        xn = ((xe + b2[e][None, :]) * SW2).reshape(MT, 128, H).transpose(1, 0, 2)
        in_maps.append({
            "xT8": np.ascontiguousarray(xT8.astype(f8)),
            "w18": np.ascontiguousarray(w18.astype(f8)),
            "w28": np.ascontiguousarray(w28.astype(f8)),
            "b1t": np.ascontiguousarray(b1t, dtype=np.float32),
            "xn": np.ascontiguousarray(xn, dtype=np.float32),
        })
    return in_maps, order


def kernel(x, task_ids, W1, b1, W2, b2, gamma, beta):
    from concourse import bass_utils

    x = np.asarray(x, dtype=np.float32)
    task_ids = np.asarray(task_ids)
    W1 = np.asarray(W1, dtype=np.float32)
    b1 = np.asarray(b1, dtype=np.float32)
    W2 = np.asarray(W2, dtype=np.float32)
    b2 = np.asarray(b2, dtype=np.float32)
    gamma = np.asarray(gamma, dtype=np.float32)
    beta = np.asarray(beta, dtype=np.float32)

    in_maps, order = _shard_inputs(x, task_ids, W1, b1, W2, b2)
    nc = _get_nc()
    res = bass_utils.run_bass_kernel_spmd(nc, in_maps, core_ids=list(range(E)))
    _CACHE["last_results"] = res

    z = np.concatenate(
        [res.results[e]["out"].transpose(1, 0, 2).reshape(C, H) for e in range(E)],
        axis=0)
    # per-expert gamma/beta (identity for this problem's inputs; applied on
    # host only when nontrivial, matching the reference's z*gamma + beta)
    if not (np.all(gamma == 1.0) and np.all(beta == 0.0)):
        blk = np.repeat(np.arange(E), C)  # reference uses capacity blocks
        z = z * gamma[blk] + beta[blk]
    out = np.empty((N, H), dtype=np.float32)
    out[order] = z
    return out


# revision 29
# speedup vs baseline: 1.0253x; 1.0253x over previous
"""MoE FFN (BertGeneration-style) on 8 TRN2 NeuronCores, expert-parallel.

Problem: 8192 tokens, expert = task_id % 8, per-expert FFN
(768 -> 3072 gelu -> 768) + residual + per-expert LayerNorm.

Strategy: routing (dispatch/combine) is a host-side permutation; each of the
8 cores runs one expert's FFN over its 1024-token block.  Both GEMMs run in
fp8 e4m3 with the PE's DoubleRow perf mode (two contraction rows packed per
partition -> K=256 per matmul, 2x fp32r throughput).  Power-of-2 scales keep
the fp8 payloads in e4m3's normal range:
  x*32, W1*4096  -> gelu dequants via activation scale 2^-17 (exact)
  W2*8192        -> cancels in LayerNorm (scale-invariant) by pre-scaling
                    the residual (x+b2)*8192 on host; no extra on-chip ops.
On-chip:
  phase 1:  hT[i, m] = gelu(psum * 2^-17 + b1) as fp8   (h transposed)
  phase 2:  y[m, h]  = sum_i hT[i, m] * W2[i, h];  z = y + 8192*(x + b2);
            LayerNorm(z) along h.
"""

import sys

if "/opt/trn_rl_repo" not in sys.path:
    sys.path.insert(0, "/opt/trn_rl_repo")

import numpy as np


def _install_axon_hooks_shim():
    """Provide antenv.axon_hooks (NTFF profiling hook) when the image's
    antenv lacks it — a thin ctypes wrapper over libaxon_pjrt.so, matching
    trn_agent_boot.trn_boot._ntff_profile_via_ctypes.  Only exercised when
    profiling is requested (BASS_TRACE); harmless otherwise."""
    import contextlib
    import ctypes
    import types

    try:
        import antenv.axon_hooks  # noqa: F401
        return
    except ImportError:
        pass
    try:
        import antenv
    except ImportError:
        return

    mod = types.ModuleType("antenv.axon_hooks")
    _state = {"hook": None, "init": False}

    def set_axon_ntff_profile_hook(h):
        _state["hook"] = h
        _state["init"] = True

    def get_axon_ntff_profile_hook():
        if _state["init"]:
            return _state["hook"]
        _state["init"] = True
        try:
            lib = ctypes.CDLL("/opt/axon/libaxon_pjrt.so")
        except OSError:
            return None
        if not hasattr(lib, "axon_start_nrt_profile"):
            return None
        lib.axon_start_nrt_profile.argtypes = [
            ctypes.POINTER(ctypes.c_int64), ctypes.c_size_t]
        lib.axon_start_nrt_profile.restype = ctypes.c_int64
        lib.axon_stop_nrt_profile.argtypes = [ctypes.c_char_p]
        lib.axon_stop_nrt_profile.restype = ctypes.c_int64

        @contextlib.contextmanager
        def _hook(output_dir, device_ids):
            import jax
            jax.devices()
            if device_ids:
                ids = (ctypes.c_int64 * len(device_ids))(*device_ids)
                rc = lib.axon_start_nrt_profile(ids, len(device_ids))
            else:
                rc = lib.axon_start_nrt_profile(None, 0)
            if rc != 0:
                raise RuntimeError(f"axon_start_nrt_profile rc={rc}")
            try:
                yield
            finally:
                n = lib.axon_stop_nrt_profile(str(output_dir).encode())
                print(f"profile: {n} file(s) written to {output_dir}")

        _state["hook"] = _hook
        return _hook

    mod.set_axon_ntff_profile_hook = set_axon_ntff_profile_hook
    mod.get_axon_ntff_profile_hook = get_axon_ntff_profile_hook
    sys.modules["antenv.axon_hooks"] = mod
    antenv.axon_hooks = mod


_install_axon_hooks_shim()

E = 8
N = 8192
H = 768
I = 3072
C = N // E        # 1024 tokens per expert/core
KT8 = H // 256    # 3   k-tiles (hidden dim, DoubleRow: 256 per matmul)
IT = I // 128     # 24  i-tiles (intermediate dim)
IT8 = I // 256    # 12  GEMM2 k-tiles (DoubleRow)
MT = C // 128     # 8   m-tiles (token dim per core)
WC = 12           # w1 DMA chunks (2 i-tiles each)

# power-of-2 fp8 scales (e4m3 max 240; values land in the normal range)
SX = 32.0
SW1 = 4096.0
SW2 = 8192.0
INV1 = 1.0 / (SX * SW1)   # dequant folded into the gelu activation

_CACHE = {}


def _build_nc(act_name="Gelu"):
    from contextlib import ExitStack

    import concourse.tile as tile
    from concourse import bacc, mybir

    f32 = mybir.dt.float32
    f8 = mybir.dt.float8e4
    AF = mybir.ActivationFunctionType
    act_fn = getattr(AF, act_name)
    ALU = mybir.AluOpType
    DR = mybir.MatmulPerfMode.DoubleRow

    nc = bacc.Bacc("TRN2", target_bir_lowering=False, debug=False, num_devices=8)

    # fp8 operands: [partition, ...] with DoubleRow pairs as an explicit dim
    xT8 = nc.dram_tensor("xT8", [128, KT8, 2, C], f8, kind="ExternalInput").ap()
    w18 = nc.dram_tensor("w18", [128, WC, 2, KT8, 2, 128], f8,
                         kind="ExternalInput").ap()
    w28 = nc.dram_tensor("w28", [128, IT8, 2, H], f8, kind="ExternalInput").ap()
    b1t = nc.dram_tensor("b1t", [128, IT], f32, kind="ExternalInput").ap()
    xn = nc.dram_tensor("xn", [128, MT, H], f32, kind="ExternalInput").ap()
    out = nc.dram_tensor("out", [128, MT, H], f32, kind="ExternalOutput").ap()

    with ExitStack() as ctx:
        tc = ctx.enter_context(tile.TileContext(nc))
        persist = ctx.enter_context(tc.tile_pool(name="persist", bufs=1))
        psum = ctx.enter_context(tc.tile_pool(name="psum", bufs=4, space="PSUM"))
        w1pool = ctx.enter_context(tc.tile_pool(name="w1s", bufs=6))
        spool = ctx.enter_context(tc.tile_pool(name="small", bufs=4))
        zpool = ctx.enter_context(tc.tile_pool(name="zs", bufs=3))

        hT = persist.tile([128, IT, C], f8, name="hT")
        w2f = [persist.tile([128, 4, 2, H], f8, name=f"w2c{j}", tag=f"w2c{j}")
               for j in range(IT8 // 4)]
        xk0a = persist.tile([128, 2, 512], f8, name="xk0a")
        xk0b = persist.tile([128, 2, 512], f8, name="xk0b")
        xk12 = persist.tile([128, 2, 2, C], f8, name="xk12")
        xnh = [persist.tile([128, 4, H], f32, name=f"xnh{j}", tag=f"xnh{j}")
               for j in range(2)]
        b1s = persist.tile([128, IT], f32, name="b1s")
        epsT = persist.tile([128, 1], f32, name="epsT")
        # PE/act warm-up scratch (zeros; results never consumed)
        wuL = persist.tile([128, 2, 128], f8, name="wuL")
        wuR = persist.tile([128, 2, 512], f8, name="wuR")
        wuO = persist.tile([128, 1], f32, name="wuO")

        nc.vector.memset(wuL, 0.0)
        nc.vector.memset(wuR, 0.0)
        nc.vector.memset(epsT, 1e-12)
        # load the Gelu act table during the startup-DMA window instead of
        # lazily at the first real activation
        nc.scalar.activation(wuO, epsT, act_fn, bias=epsT)

        def rhs1(t, half):
            if t == 0:
                return xk0a if half == 0 else xk0b
            return xk12[:, t - 1][:, :, half * 512:(half + 1) * 512]

        # ---- startup DMAs: both queues stream the PE-critical chunks
        # first (w1 even chunks on sync, odd on gpsimd); w2/xn queue behind
        # them so ring contention can't starve the phase-1 weight stream
        w1ts = {}

        # The w1 chunk stream — PE-critical — rides the sync hardware DMA
        # ring alone (issue rate 0.65us/chunk vs 2.6us/chunk consumption).
        # The gpsimd ring is Q7-software-driven (~2-3us per descriptor) and
        # gets the early-small (xk, b1) and high-slack (w2) transfers.
        def load_w1(c):
            w1t = w1pool.tile([128, 2, KT8, 2, 128], f8, name="w1t", tag="w1t")
            nc.sync.dma_start(out=w1t, in_=w18[:, c])
            w1ts[c] = w1t

        nc.gpsimd.dma_start(out=xk0a, in_=xT8[:, 0, :, 0:512])
        load_w1(0)
        load_w1(1)
        nc.gpsimd.dma_start(out=xk0b, in_=xT8[:, 0, :, 512:1024])
        nc.gpsimd.dma_start(out=xk12, in_=xT8[:, 1:3])
        nc.gpsimd.dma_start(out=b1s, in_=b1t)
        load_w1(2)
        load_w1(3)

        # ramp the PE clock out of its low pstate while the startup DMAs
        # are in flight (zero-input matmuls; psum result unread)
        wuP = psum.tile([128, C], f32, name="ph", tag="pt")
        for _ in range(2):
            nc.tensor.matmul(wuP[:, 0:512], lhsT=wuL, rhs=wuR, start=True,
                             stop=True, perf_mode=DR)

        # ---- phase 1: hT = gelu((W1.T @ xT) * 2^-17 + b1) as fp8 ----
        # warm-up block: first NW i-tiles processed kt-major so the PE can
        # start on xk chunk 0 while chunks 1-2 are still in flight
        NW = 3
        load_w1(4)
        load_w1(5)
        phs = [psum.tile([128, C], f32, name="ph", tag="pt") for _ in range(NW)]
        for t in range(KT8):
            for it in range(NW):
                lhsT = w1ts[it // 2][:, it % 2, t]
                for half in range(2):
                    nc.tensor.matmul(
                        phs[it][:, half * 512:(half + 1) * 512],
                        lhsT=lhsT,
                        rhs=rhs1(t, half),
                        start=(t == 0),
                        stop=(t == KT8 - 1),
                        perf_mode=DR,
                    )
        for it in range(NW):
            nc.scalar.activation(hT[:, it, :], phs[it], act_fn,
                                 bias=b1s[:, it:it + 1], scale=INV1)

        for it in range(NW, IT):
            if it % 2 == 0 and it // 2 + 4 < WC:
                load_w1(it // 2 + 4)
            ph = psum.tile([128, C], f32, name="ph", tag="pt")
            lhs_c = w1ts[it // 2]
            for t in range(KT8):
                lhsT = lhs_c[:, it % 2, t]
                for half in range(2):
                    nc.tensor.matmul(
                        ph[:, half * 512:(half + 1) * 512],
                        lhsT=lhsT,
                        rhs=rhs1(t, half),
                        start=(t == 0),
                        stop=(t == KT8 - 1),
                        perf_mode=DR,
                    )
            nc.scalar.activation(hT[:, it, :], ph, act_fn,
                                 bias=b1s[:, it:it + 1], scale=INV1)

        # w2/xn have tens of us of slack before phase 2: w2 behind the xk
        # stream on the slow gpsimd ring, xn behind the w1 chunks on sync
        for j in range(IT8 // 4):
            nc.gpsimd.dma_start(out=w2f[j], in_=w28[:, 4 * j:4 * j + 4])
        for j in range(2):
            nc.sync.dma_start(out=xnh[j], in_=xn[:, 4 * j:4 * j + 4])

        # ---- phase 2: y = hT.T @ W2; z = y + 8192*(x+b2); LayerNorm ----
        for mt in range(MT):
            py = psum.tile([128, C], f32, name="py", tag="pt")
            for t in range(IT8):
                lhsT = hT[:, 2 * t:2 * t + 2, mt * 128:(mt + 1) * 128]
                rhsc = w2f[t // 4][:, t % 4]
                nc.tensor.matmul(
                    py[:, 0:512], lhsT=lhsT, rhs=rhsc[:, :, 0:512],
                    start=(t == 0), stop=(t == IT8 - 1), perf_mode=DR)
                nc.tensor.matmul(
                    py[:, 512:768], lhsT=lhsT, rhs=rhsc[:, :, 512:768],
                    start=(t == 0), stop=(t == IT8 - 1), perf_mode=DR)
            # residual + LayerNorm (gpsimd cannot read PSUM, so the add
            # stays on vector; the normalize halves split across engines)
            z = zpool.tile([128, H], f32, name="z", tag="z")
            xr = xnh[mt // 4][:, mt % 4]
            HH = H // 2
            nc.vector.tensor_add(z, py[:, 0:H], xr)
            stats = spool.tile([128, 2, 6], f32, name="stats", tag="stats")
            nc.vector.bn_stats(stats[:, 0], z[:, 0:HH])
            nc.vector.bn_stats(stats[:, 1], z[:, HH:H])
            mv = spool.tile([128, 2], f32, name="mv", tag="mv")
            nc.vector.bn_aggr(mv, stats)
            rstd = spool.tile([128, 1], f32, name="rstd", tag="rstd")
            nc.scalar.activation(rstd, mv[:, 1:2], AF.Sqrt, bias=epsT)
            nc.vector.reciprocal(out=rstd, in_=rstd)
            for h0 in (0, HH):
                sl = slice(h0, h0 + HH)
                nc.vector.tensor_scalar(
                    out=z[:, sl], in0=z[:, sl], scalar1=mv[:, 0:1],
                    scalar2=rstd, op0=ALU.subtract, op1=ALU.mult)
                nc.scalar.dma_start(out=out[:, mt, sl], in_=z[:, sl])

    nc.compile()
    return nc


def _get_nc(act_name="Gelu"):
    key = ("nc", act_name)
    if key not in _CACHE:
        _CACHE[key] = _build_nc(act_name)
    return _CACHE[key]


def _shard_inputs(x, task_ids, W1, b1, W2, b2):
    """Host-side dispatch: stable-sort tokens by expert id, chunk into E
    equal capacity-C blocks (exactly the reference's xs = x[order].reshape),
    then pack/quantize the fp8 operand layouts."""
    import ml_dtypes

    f8 = ml_dtypes.float8_e4m3
    expert = (task_ids.astype(np.int64) % E).astype(np.int32)
    order = np.argsort(expert, kind="stable")
    xs = x[order]
    in_maps = []
    for e in range(E):
        xe = xs[e * C:(e + 1) * C]                       # [C, H]
        # xT8[p, t, j, c] = xe[c, t*256 + j*128 + p] * SX
        xT8 = (xe.T * SX).reshape(KT8, 2, 128, C).transpose(2, 0, 1, 3)
        # w18[p, chunk, itc, t, j, m] = W1[t*256+j*128+p, (chunk*2+itc)*128+m]
        w18 = (W1[e] * SW1).reshape(KT8, 2, 128, WC, 2, 128).transpose(
            2, 3, 4, 0, 1, 5)
        # w28[p, t, j, h] = W2[t*256+j*128+p, h] * SW2
        w28 = (W2[e] * SW2).reshape(IT8, 2, 128, H).transpose(2, 0, 1, 3)
        b1t = b1[e].reshape(IT, 128).T
        xn = ((xe + b2[e][None, :]) * SW2).reshape(MT, 128, H).transpose(1, 0, 2)
        in_maps.append({
            "xT8": np.ascontiguousarray(xT8.astype(f8)),
            "w18": np.ascontiguousarray(w18.astype(f8)),
            "w28": np.ascontiguousarray(w28.astype(f8)),
            "b1t": np.ascontiguousarray(b1t, dtype=np.float32),
            "xn": np.ascontiguousarray(xn, dtype=np.float32),
        })
    return in_maps, order


def kernel(x, task_ids, W1, b1, W2, b2, gamma, beta):
    from concourse import bass_utils

    x = np.asarray(x, dtype=np.float32)
    task_ids = np.asarray(task_ids)
    W1 = np.asarray(W1, dtype=np.float32)
    b1 = np.asarray(b1, dtype=np.float32)
    W2 = np.asarray(W2, dtype=np.float32)
    b2 = np.asarray(b2, dtype=np.float32)
    gamma = np.asarray(gamma, dtype=np.float32)
    beta = np.asarray(beta, dtype=np.float32)

    in_maps, order = _shard_inputs(x, task_ids, W1, b1, W2, b2)
    nc = _get_nc()
    res = bass_utils.run_bass_kernel_spmd(nc, in_maps, core_ids=list(range(E)))
    _CACHE["last_results"] = res

    z = np.concatenate(
        [res.results[e]["out"].transpose(1, 0, 2).reshape(C, H) for e in range(E)],
        axis=0)
    # per-expert gamma/beta (identity for this problem's inputs; applied on
    # host only when nontrivial, matching the reference's z*gamma + beta)
    if not (np.all(gamma == 1.0) and np.all(beta == 0.0)):
        blk = np.repeat(np.arange(E), C)  # reference uses capacity blocks
        z = z * gamma[blk] + beta[blk]
    out = np.empty((N, H), dtype=np.float32)
    out[order] = z
    return out
